# revision 28
# baseline (speedup 1.0000x reference)
"""Trainium2 Bass kernel for the BDH reasoner step (topk masking problem).

Reference computation (see problem statement):
    h         = relu(x @ W_enc.T + b_enc)            [1, 8192]
    kth       = 409-th largest of h
    h_sparse  = where(h >= kth, h, 0)
    new_state = tanh(h_sparse @ synapses + state)    [1, 8192]
    logits    = new_state @ W_cls.T + b_cls          [1, 2]
    updated   = synapses + lr * h_sparse.T @ h_sparse
    new_syn   = updated / ||updated||_F              [8192, 8192]

Sharding (8 cores): core c owns hidden rows/cols [c*1024, (c+1)*1024):
  - W_enc row shard -> local h slice, AllGather -> full h
  - top-k threshold found by fixed-count float bisection (exact: converges
    to the k-th value at fp32 resolution)
  - synapses column shard: z_c = h_sparse @ syn[:, cols] and the outer
    product slice computed locally
  - ||updated||^2 = ||syn||^2 + 2*lr*<z, hs> + lr^2*||hs||^4, reduced with
    one 4-float AllReduce (which also carries the logits partials)
  - output slice written as (syn + s*lr*outer) with s = 1/norm folded in
"""

import os
import sys
import threading

import numpy as np

sys.path.insert(0, "/opt/trn_rl_repo")

from concourse import bacc, bass, bass_isa, masks, mybir, tile
from concourse.bass_utils import run_bass_kernel_spmd

F32 = mybir.dt.float32
F32R = mybir.dt.float32r
AF = mybir.ActivationFunctionType
ALU = mybir.AluOpType
AX = mybir.AxisListType

NCORES = 8
D = 4096          # input dim
H = 8192          # hidden dim
COLS = H // NCORES  # 1024 columns (and h rows) per core
K = 409
LR = 1e-3
NROWT = H // 128  # 64 row tiles of the synapses shard
HELD = 34         # row tiles kept in SBUF between pass 1 and pass 2
NITER = 30        # bisection iterations
RG = [list(range(NCORES))]
STAGE = int(os.environ.get("KSTAGE", "9"))  # truncate build for hang bisection
NOCC = bool(os.environ.get("KNOCC"))  # stub collectives (single-core timing model)

# z / outer-product matmuls run as float32r (fast PE path); everything that
# feeds the top-k mask or the dominant output term stays exact fp32.
MM_DT = F32R


def _mmcast(ap):
    return ap.bitcast(MM_DT)


def build_nc():
    nc = bacc.Bacc(
        "TRN2",
        target_bir_lowering=False,
        debug=False,
        num_devices=NCORES,
    )

    x_d = nc.dram_tensor("x_in", [1, D], F32, kind="ExternalInput").ap()
    wenc_d = nc.dram_tensor("wenc", [COLS, D], F32, kind="ExternalInput").ap()
    benc_d = nc.dram_tensor("benc", [1, COLS], F32, kind="ExternalInput").ap()
    syn_d = nc.dram_tensor("syn", [H, COLS], F32, kind="ExternalInput").ap()
    state_d = nc.dram_tensor("state_in", [1, COLS], F32, kind="ExternalInput").ap()
    wcls_d = nc.dram_tensor("wcls", [2, COLS], F32, kind="ExternalInput").ap()
    bcls_d = nc.dram_tensor("bcls", [1, 2], F32, kind="ExternalInput").ap()

    logits_d = nc.dram_tensor("out_logits", [1, 2], F32, kind="ExternalOutput").ap()
    nstate_d = nc.dram_tensor("out_state", [1, COLS], F32, kind="ExternalOutput").ap()
    nsyn_d = nc.dram_tensor("out_syn", [H, COLS], F32, kind="ExternalOutput").ap()

    with tile.TileContext(nc) as tc:
        _build_body(nc, tc, x_d, wenc_d, benc_d, syn_d, state_d, wcls_d, bcls_d,
                    logits_d, nstate_d, nsyn_d)
    nc.compile()
    return nc


def _build_body(nc, tc, x_d, wenc_d, benc_d, syn_d, state_d, wcls_d, bcls_d,
                logits_d, nstate_d, nsyn_d):
    from contextlib import ExitStack

    ctx = ExitStack()
    with ctx:
        persist = ctx.enter_context(tc.tile_pool(name="persist", bufs=1))
        dram = ctx.enter_context(tc.tile_pool(name="dram", bufs=1, space="DRAM"))
        pshold = ctx.enter_context(tc.tile_pool(name="pshold", bufs=1, space="PSUM"))

        # ---- persistent small tiles ----
        ones_sb = persist.tile([128, 128], F32)
        nc.vector.memset(ones_sb[:], 1.0)
        ident = persist.tile([128, 128], F32)
        masks.make_identity(nc, ident[:])

        hc = persist.tile([128, 8], F32)        # local h slice, [p, t] = h[c*1024 + t*128 + p]
        h_all = persist.tile([128, 64], F32)    # full h, [p, f] = h[f*128 + p]
        hT = persist.tile([64, 128], F32)       # full h, [f, p] = h[f*128 + p]
        hs_all = persist.tile([128, 64], F32R)  # masked h (z lhsT chunks)
        hs_flat = persist.tile([33, H // 2], F32R)  # masked h rows on partitions {0,32}
        hs_cols = persist.tile([1, COLS], F32)  # masked local slice (outer rhs)
        hs_cols_s = persist.tile([33, COLS], F32R)  # rows {0,32} hold the same data
        sq_cols = persist.tile([128, NROWT], F32)

        lo = persist.tile([128, 1], F32)
        hi = persist.tile([128, 1], F32)
        tt = persist.tile([128, 1], F32)
        tsum = persist.tile([128, 1], F32)
        bfl = persist.tile([128, 1], F32)
        nbfl = persist.tile([128, 1], F32)
        cmp = persist.tile([128, 64], F32)
        cntp = persist.tile([128, 1], F32)
        mx = persist.tile([128, 1], F32)
        mx2 = persist.tile([128, 1], F32)
        hsq_p = persist.tile([128, 1], F32)
        hsq_t = persist.tile([128, 1], F32)
        sq_p = persist.tile([128, 1], F32)
        s_b = persist.tile([128, 1], F32)

        pk = persist.tile([1, 4], F32)          # [sqsum, dot, logit0, logit1]
        ar_sb = persist.tile([1, 4], F32)
        u1 = persist.tile([1, 1], F32)
        u2 = persist.tile([1, 1], F32)
        u3 = persist.tile([1, 1], F32)
        u4 = persist.tile([1, 1], F32)
        nrm = persist.tile([1, 1], F32)
        s_sc = persist.tile([1, 1], F32)
        s_lr = persist.tile([1, 1], F32)
        lgout = persist.tile([1, 2], F32)
        bcls_sb = persist.tile([1, 2], F32)

        cps = pshold.tile([128, 1], F32)        # count broadcast psum

        # ================= encoder =================
        with tc.tile_pool(name="encb", bufs=1) as encb, \
             tc.tile_pool(name="encw", bufs=3) as encw, \
             tc.tile_pool(name="encj", bufs=2) as encj:
            xrow = encb.tile([1, D], F32)
            nc.sync.dma_start(out=xrow[:], in_=x_d[:, :])
            xb = encb.tile([128, D], F32)
            with tc.tile_pool(name="psX", bufs=2, space="PSUM") as psX:
                for k in range(D // 512):
                    pxb = psX.tile([128, 512], F32, tag="pxb", name=f"pxb{k}")
                    nc.tensor.matmul(pxb[:], ones_sb[0:1, :],
                                     xrow[0:1, k * 512:(k + 1) * 512],
                                     start=True, stop=True)
                    nc.vector.tensor_copy(xb[:, k * 512:(k + 1) * 512], pxb[:])

            benc_sb = encb.tile([128, 8], F32)
            benc8 = encb.tile([8, 128], F32)
            nc.sync.dma_start(
                out=benc8[:], in_=benc_d[0].rearrange("(t p) -> t p", p=128))
            with tc.tile_pool(name="psB", bufs=1, space="PSUM") as psB:
                pb = psB.tile([128, 8], F32)
                nc.tensor.transpose(pb[:], benc8[:], ident[:8, :8])
                nc.vector.tensor_copy(benc_sb[:], pb[:])

            HALF = D // 2
            hparts = encb.tile([128, 16], F32)
            for t in range(8):
                for half in range(2):
                    wt = encw.tile([128, HALF], F32, tag="wt")
                    nc.scalar.dma_start(
                        out=wt[:],
                        in_=wenc_d[t * 128:(t + 1) * 128,
                                   half * HALF:(half + 1) * HALF],
                    )
                    junk = encj.tile([128, HALF], F32, tag="junk")
                    nc.vector.tensor_mul(
                        out=junk[:], in0=wt[:],
                        in1=xb[:, half * HALF:(half + 1) * HALF])
                    nc.vector.reduce_sum(
                        hparts[:, 2 * t + half:2 * t + half + 1], junk[:],
                        axis=AX.X)
            nc.vector.tensor_tensor(
                out=hc[:], in0=hparts[:, 0:16:2], in1=hparts[:, 1:16:2],
                op=ALU.add)
            # bias + relu
            nc.vector.tensor_tensor(out=hc[:], in0=hc[:], in1=benc_sb[:], op=ALU.add)
            nc.vector.tensor_scalar_max(hc[:], hc[:], 0.0)

        if STAGE < 2:
            nc.sync.dma_start(out=nstate_d[0:1, 0:8], in_=hc[0:1, :])
            return

        # ================= allgather h =================
        hg_in = dram.tile([COLS], F32)
        hg_out = dram.tile([H], F32, addr_space="Shared")
        with tc.tile_pool(name="psG", bufs=1, space="PSUM") as psG:
            pg = psG.tile([8, 128], F32)
            nc.tensor.transpose(pg[:], hc[:], ident[:])
            hc8 = persist.tile([8, 128], F32)
            nc.vector.tensor_copy(hc8[:], pg[:])
        nc.gpsimd.dma_start(out=hg_in.rearrange("(t p) -> t p", p=128), in_=hc8[:])
        if NOCC:
            nc.gpsimd.dma_start(
                out=hg_out.rearrange("(c n) -> c n", n=COLS),
                in_=hg_in.unsqueeze(0).broadcast_to((NCORES, COLS)))
        else:
            nc.gpsimd.collective_compute(
                "AllGather",
                ALU.bypass,
                replica_groups=RG,
                ins=[hg_in.opt()],
                outs=[hg_out.opt()],
            )
        nc.sync.dma_start(out=hT[:], in_=hg_out.rearrange("(f p) -> f p", p=128))

        with tc.tile_pool(name="psA", bufs=1, space="PSUM") as psA:
            ps_h = psA.tile([128, 64], F32)
            nc.tensor.transpose(ps_h[:], hT[:], ident[:64, :64])
            nc.vector.tensor_copy(h_all[:], ps_h[:])

            # ================= top-k threshold (bisection) =================
            nc.vector.reduce_max(mx[:], h_all[:], axis=AX.X)
            ps_t = psA.tile([1, 128], F32)
            nc.tensor.transpose(ps_t[:], mx[:], ident[:])
            mxrow = persist.tile([1, 128], F32)
            nc.vector.tensor_copy(mxrow[:], ps_t[:])
            mxs = persist.tile([1, 1], F32)
            nc.vector.reduce_max(mxs[:], mxrow[:], axis=AX.X)
            ps_b = psA.tile([128, 1], F32)
            nc.tensor.matmul(ps_b[:], ones_sb[0:1, :], mxs[:], start=True, stop=True)
            nc.vector.tensor_copy(mx2[:], ps_b[:])
            nc.vector.tensor_scalar_add(hi[:], mx2[:], 1.0)
            nc.vector.memset(lo[:], 0.0)
            nc.vector.tensor_scalar_mul(tt[:], hi[:], 0.5)

            kf = float(K)
            for it in range(NITER):
                nc.vector.tensor_scalar(
                    out=cmp[:], in0=h_all[:], scalar1=tt[:, 0:1], scalar2=None,
                    op0=ALU.is_ge, op1=ALU.add, accum_out=cntp[:])
                nc.tensor.matmul(cps[:], ones_sb[:], cntp[:], start=True, stop=True)
                nc.vector.tensor_scalar(
                    out=bfl[:], in0=cps[:], scalar1=kf, scalar2=None, op0=ALU.is_ge)
                nc.vector.tensor_scalar(
                    out=nbfl[:], in0=cps[:], scalar1=kf, scalar2=None, op0=ALU.is_lt)
                nc.vector.copy_predicated(lo[:], bfl[:].bitcast(mybir.dt.int32), tt[:])
                nc.vector.copy_predicated(hi[:], nbfl[:].bitcast(mybir.dt.int32), tt[:])
                nc.vector.tensor_tensor(out=tsum[:], in0=lo[:], in1=hi[:], op=ALU.add)
                nc.vector.tensor_scalar_mul(tt[:], tsum[:], 0.5)

            # ================= masks =================
            nc.vector.tensor_scalar(
                out=cmp[:], in0=h_all[:], scalar1=lo[:, 0:1], scalar2=None,
                op0=ALU.is_ge)
            nc.vector.tensor_tensor(out=hs_all[:], in0=h_all[:], in1=cmp[:], op=ALU.mult)

            cmpT = persist.tile([64, 128], F32)
            hsT = persist.tile([64, 128], F32)
            nc.vector.tensor_scalar(
                out=cmpT[:], in0=hT[:], scalar1=lo[0:64, 0:1], scalar2=None,
                op0=ALU.is_ge)
            nc.vector.tensor_tensor(out=hsT[:], in0=hT[:], in1=cmpT[:], op=ALU.mult)
            # flatten to partition 0 via DRAM: hs_flat[0, f*128 + p] = hsT[f, p]
            hsf_dram = dram.tile([H], F32)
            nc.gpsimd.dma_start(
                out=hsf_dram.rearrange("(f p) -> f p", p=128), in_=hsT[:])
            nc.gpsimd.dma_start(out=hs_flat[0:1, :],
                                in_=hsf_dram[0:H // 2].unsqueeze(0).bitcast(F32R))
            nc.gpsimd.dma_start(out=hs_flat[32:33, :],
                                in_=hsf_dram[H // 2:].unsqueeze(0).bitcast(F32R))

            cmpc = persist.tile([128, 8], F32)
            hsc = persist.tile([128, 8], F32)
            nc.vector.tensor_scalar(
                out=cmpc[:], in0=hc[:], scalar1=lo[:, 0:1], scalar2=None,
                op0=ALU.is_ge)
            nc.vector.tensor_tensor(out=hsc[:], in0=hc[:], in1=cmpc[:], op=ALU.mult)
            # local cols on partition 0 via DRAM: hs_cols[0, t*128 + p] = hsc[p, t]
            ph = psA.tile([8, 128], F32)
            nc.tensor.transpose(ph[:], hsc[:], ident[:])
            hsc8 = persist.tile([8, 128], F32)
            nc.vector.tensor_copy(hsc8[:], ph[:])
            hsc_dram = dram.tile([COLS], F32)
            nc.gpsimd.dma_start(
                out=hsc_dram.rearrange("(t p) -> t p", p=128), in_=hsc8[:])
            nc.gpsimd.dma_start(out=hs_cols[0:1, :], in_=hsc_dram.unsqueeze(0))

            # ||hs||^2 per partition (for the norm formula)
            junk5 = persist.tile([128, 64], F32)
            nc.vector.tensor_mul(
                out=junk5[:], in0=hs_all[:].bitcast(F32),
                in1=hs_all[:].bitcast(F32))
            nc.vector.reduce_sum(hsq_p[:], junk5[:], axis=AX.X)
            ps_hq = psA.tile([128, 1], F32)
            nc.tensor.matmul(ps_hq[:], ones_sb[:], hsq_p[:], start=True, stop=True)
            nc.vector.tensor_copy(hsq_t[:], ps_hq[:])

        if STAGE < 3:
            nc.sync.dma_start(out=nstate_d[0:1, 0:1], in_=lo[0:1, :])
            return

        # ================= pass 1: stream synapses =================
        held_tiles = []
        synheld = ctx.enter_context(tc.tile_pool(name="synheld", bufs=1))
        with tc.tile_pool(name="synstream", bufs=3) as synstream, \
             tc.tile_pool(name="sqjunk", bufs=2, space="PSUM") as sqjunk, \
             tc.tile_pool(name="p1s", bufs=1) as p1s, \
             tc.tile_pool(name="psZ", bufs=1, space="PSUM") as psZ:
            zps = psZ.tile([1, COLS], F32)
            state_sb = p1s.tile([1, COLS], F32)
            zsb = p1s.tile([1, COLS], F32)
            ns = p1s.tile([1, COLS], F32)
            wcls0 = p1s.tile([1, COLS], F32)
            wcls1 = p1s.tile([1, COLS], F32)
            for t in range(NROWT):
                if t < HELD:
                    st = synheld.tile([128, COLS], F32R, tag=f"held{t}", bufs=1,
                                      name=f"held{t}")
                    held_tiles.append(st)
                else:
                    st = synstream.tile([128, COLS], F32R, tag="stream",
                                        name=f"syns{t}")
                nc.sync.dma_start(
                    out=st[:], in_=syn_d[t * 128:(t + 1) * 128, :].bitcast(F32R))
                jq = sqjunk.tile([128, COLS], F32, tag="sq", name=f"sqj{t}")
                nc.scalar.activation(
                    jq[:], st[:].bitcast(F32), AF.Square,
                    accum_out=sq_cols[:, t:t + 1])
                nc.tensor.matmul(
                    zps[0:1, 0:512], hs_all[:, t:t + 1], st[:, 0:512],
                    start=(t == 0), stop=(t == NROWT - 1))
                nc.tensor.matmul(
                    zps[0:1, 512:1024], hs_all[:, t:t + 1], st[:, 512:1024],
                    start=(t == 0), stop=(t == NROWT - 1))

            # ---- scalars for the norm + outputs ----
            nc.sync.dma_start(out=state_sb[:], in_=state_d[:, :])
            nc.sync.dma_start(out=wcls0[:], in_=wcls_d[0:1, :])
            nc.sync.dma_start(out=wcls1[:], in_=wcls_d[1:2, :])
            nc.sync.dma_start(out=bcls_sb[:], in_=bcls_d[:, :])

            # new_state slice
            nc.vector.tensor_tensor(out=zsb[:], in0=zps[:], in1=state_sb[:], op=ALU.add)
            nc.scalar.activation(ns[:], zsb[:], AF.Tanh)
            nc.sync.dma_start(out=nstate_d[:, :], in_=ns[:])

            # dot(z, hs) partial  -> pk[0,1]
            junk6 = p1s.tile([1, COLS], F32)
            nc.vector.tensor_mul(out=junk6[:], in0=zps[:], in1=hs_cols[:])
            nc.vector.reduce_sum(pk[0:1, 1:2], junk6[:], axis=AX.X)
            # logits partials -> pk[0,2:4]
            nc.vector.tensor_mul(out=junk6[:], in0=ns[:], in1=wcls0[:])
            nc.vector.reduce_sum(pk[0:1, 2:3], junk6[:], axis=AX.X)
            nc.vector.tensor_mul(out=junk6[:], in0=ns[:], in1=wcls1[:])
            nc.vector.reduce_sum(pk[0:1, 3:4], junk6[:], axis=AX.X)
            # ||syn_c||^2 partial -> pk[0,0]
            nc.vector.reduce_sum(sq_p[:], sq_cols[:], axis=AX.X)
            nc.tensor.matmul(cps[:], ones_sb[:], sq_p[:], start=True, stop=True)
            nc.vector.tensor_copy(pk[0:1, 0:1], cps[0:1, 0:1])

        # ================= allreduce the 4 scalars =================
        ar_in = dram.tile([1, 4], F32)
        ar_out = dram.tile([1, 4], F32, addr_space="Shared")
        nc.gpsimd.dma_start(out=ar_in[:], in_=pk[:])
        if NOCC:
            nc.gpsimd.dma_start(out=ar_out[:], in_=ar_in[:])
        else:
            nc.gpsimd.collective_compute(
                "AllReduce",
                ALU.add,
                replica_groups=RG,
                ins=[ar_in.opt()],
                outs=[ar_out.opt()],
            )
        nc.gpsimd.dma_start(out=ar_sb[:], in_=ar_out[:])

        # s = 1/sqrt(sq + 2*lr*dot + lr^2 * (||hs||^2)^2)
        nc.vector.tensor_scalar_mul(u1[:], ar_sb[0:1, 1:2], 2.0 * LR)
        nc.vector.tensor_tensor(out=u2[:], in0=u1[:], in1=ar_sb[0:1, 0:1], op=ALU.add)
        nc.vector.tensor_tensor(out=u3[:], in0=hsq_t[0:1, 0:1], in1=hsq_t[0:1, 0:1],
                                op=ALU.mult)
        nc.vector.tensor_scalar_mul(u3[:], u3[:], LR * LR)
        nc.vector.tensor_tensor(out=u4[:], in0=u2[:], in1=u3[:], op=ALU.add)
        nc.scalar.sqrt(nrm[:], u4[:])
        nc.vector.reciprocal(s_sc[:], nrm[:])
        nc.tensor.matmul(cps[:], ones_sb[0:1, :], s_sc[:], start=True, stop=True)
        nc.vector.tensor_copy(s_b[:], cps[:])
        nc.vector.tensor_scalar_mul(s_lr[:], s_sc[:], LR)
        nc.vector.tensor_scalar(
            out=hs_cols_s[0:1, :], in0=hs_cols[:], scalar1=s_lr[0:1, 0:1],
            scalar2=None, op0=ALU.mult)
        nc.gpsimd.dma_start(out=hs_cols_s[32:33, :], in_=hs_cols_s[0:1, :])

        # logits output (every core computes it; host picks core 0)
        nc.vector.tensor_tensor(out=lgout[:], in0=ar_sb[0:1, 2:4], in1=bcls_sb[:],
                                op=ALU.add)
        nc.sync.dma_start(out=logits_d[:, :], in_=lgout[:])

        if STAGE < 4:
            return

        # ================= pass 2: outputs =================
        with tc.tile_pool(name="syn2", bufs=4) as syn2, \
             tc.tile_pool(name="sspool", bufs=3) as sspool, \
             tc.tile_pool(name="psO", bufs=3, space="PSUM") as psO:
            order = list(range(HELD, NROWT)) + list(range(HELD))
            for t in order:
                if t < HELD:
                    st = held_tiles[t]
                else:
                    st = syn2.tile([128, COLS], F32R, tag="re", name=f"synr{t}")
                    nc.sync.dma_start(
                        out=st[:],
                        in_=syn_d[t * 128:(t + 1) * 128, :].bitcast(F32R))
                fr = 0 if t < 32 else 32
                fo = (t % 32) * 128
                po = psO.tile([128, COLS], F32, tag="po", name=f"po{t}")
                nc.tensor.matmul(
                    po[:, 0:512], hs_flat[fr:fr + 1, fo:fo + 128],
                    hs_cols_s[fr:fr + 1, 0:512], start=True, stop=True)
                nc.tensor.matmul(
                    po[:, 512:1024], hs_flat[fr:fr + 1, fo:fo + 128],
                    hs_cols_s[fr:fr + 1, 512:1024], start=True, stop=True)
                ss = sspool.tile([128, COLS], F32, tag="ss", name=f"ss{t}")
                nc.vector.scalar_tensor_tensor(
                    out=ss[:], in0=st[:].bitcast(F32), scalar=s_b[:, 0:1],
                    in1=po[:], op0=ALU.mult, op1=ALU.add)
                # scalar-engine HWDGE queue: keeps the s-gated output stores
                # from head-blocking the pass-2 re-read prefetches on sync
                nc.scalar.dma_start(out=nsyn_d[t * 128:(t + 1) * 128, :], in_=ss[:])


_NC_LOCK = threading.Lock()
_NC_CACHE = {}


def _get_nc():
    with _NC_LOCK:
        if "nc" not in _NC_CACHE:
            _NC_CACHE["nc"] = build_nc()
        return _NC_CACHE["nc"]


def _make_in_maps(inputs):
    x = np.ascontiguousarray(np.asarray(inputs["x"], dtype=np.float32))
    W_enc = np.asarray(inputs["W_enc"], dtype=np.float32)
    b_enc = np.asarray(inputs["b_enc"], dtype=np.float32).reshape(1, H)
    synapses = np.asarray(inputs["synapses"], dtype=np.float32)
    state = np.asarray(inputs["state"], dtype=np.float32).reshape(1, H)
    W_cls = np.asarray(inputs["W_cls"], dtype=np.float32)
    b_cls = np.asarray(inputs["b_cls"], dtype=np.float32).reshape(1, 2)

    in_maps = []
    for c in range(NCORES):
        sl = slice(c * COLS, (c + 1) * COLS)
        in_maps.append({
            "x_in": x,
            "wenc": np.ascontiguousarray(W_enc[sl, :]),
            "benc": np.ascontiguousarray(b_enc[:, sl]),
            "syn": np.ascontiguousarray(synapses[:, sl]),
            "state_in": np.ascontiguousarray(state[:, sl]),
            "wcls": np.ascontiguousarray(W_cls[:, sl]),
            "bcls": b_cls,
        })
    return in_maps


def _assemble(res):
    logits = np.asarray(res[0]["out_logits"]).reshape(1, 2)
    new_state = np.concatenate(
        [np.asarray(res[c]["out_state"]).reshape(1, COLS) for c in range(NCORES)],
        axis=1)
    new_syn = np.concatenate(
        [np.asarray(res[c]["out_syn"]).reshape(H, COLS) for c in range(NCORES)],
        axis=1)
    return logits, new_state, new_syn


def kernel(x, W_enc, b_enc, synapses, state, W_cls, b_cls):
    inputs = dict(x=x, W_enc=W_enc, b_enc=b_enc, synapses=synapses,
                  state=state, W_cls=W_cls, b_cls=b_cls)
    in_maps = _make_in_maps(inputs)
    nc = _get_nc()
    res = run_bass_kernel_spmd(nc, in_maps, list(range(NCORES))).results
    return _assemble(res)
